# revision 42
# baseline (speedup 1.0000x reference)
"""CIGLoss (segment_reduce) Trainium2 kernel.

Strategy (data-parallel over batch, per the sharding hint):
  - Each of the 8 NeuronCores owns one image and that image's pixel list
    (segments are image-local: seg // 500 == image).
  - Host-side sharding packs each image's ~500 segments into a
    [128 partitions, NSLOT slots, L] padded grid (one whole segment per
    slot), values cast to fp16/fp8 (loss tolerance 2e-2 >> cast error).
    Pad entries are 0.  The value lookup input[b,0,row,col] happens
    during host packing (this toolchain's walrus mis-lowers per-element
    indirect DMA — verified by hardware probes in a previous session).
  - Per-segment counts are metadata (a function of seg_ids only); the
    host ships -w = -1/max(count,1) as a tiny f32 tensor and keeps
    w/npad for the final host-side combine.
  - On device, Sum_real |v-m| == 2*Sum_real relu(v-m) (real deviations
    sum to ~0), and pads (v=0) contribute relu(-m) each:
        sums_s  = accum_add(v_s)          ACT Copy-accum / DVE TS-accum
        negmean = -sums*w                 tiny DVE op
        R_s     = accum_add(relu(v_s + negmean))   ACT Relu-accum /
                                                   fused DVE STT-accum
    All accum ops run at 1x, so the 8 big passes are split evenly
    between the ACT and DVE engines (each slot's sum and dev on
    opposite engines), with single-writer-engine sum tiles and
    consumers emitted right after producers (cross-engine deps are
    tracked at tile granularity by emission position).  Both engines
    accumulate [R | negmean] into one shared result tile that is
    DMA'd out raw, so there is no device tail after the last dev op.
  - Host applies contrib = 2*w*(R - npad*relu(negmean)) per slot, sums
    the 8 cores' partials, and divides by B.
"""

import numpy as np

_NUM_PATHS = 4000
_P = 128  # SBUF partitions


def _build_nc(nslot: int, Ls: tuple, vdt: str, sums_eng: tuple, dev_eng: tuple,
              split_dma: bool):
    import concourse.bacc as bacc
    import concourse.tile as tile
    from concourse import mybir

    f32 = mybir.dt.float32
    fv = {"f16": mybir.dt.float16, "f8": mybir.dt.float8e4}[vdt]
    f16 = mybir.dt.float16
    Alu = mybir.AluOpType
    Act = mybir.ActivationFunctionType
    FREE = sum(Ls)
    OFF = [0]
    for l in Ls:
        OFF.append(OFF[-1] + l)
    LMAX = max(Ls)

    nc = bacc.Bacc("TRN2", debug=False)
    v_d = nc.dram_tensor("vP", [_P, FREE], fv, kind="ExternalInput")
    sm_d = nc.dram_tensor("smP", [_P, nslot], f32, kind="ExternalInput")
    out_d = nc.dram_tensor("out", [_P, 2 * nslot], f32, kind="ExternalOutput")

    assert nslot == 4 and sums_eng == ("act", "dve", "dve", "act") \
        and dev_eng == ("dve", "act", "act", "dve")
    with tile.TileContext(nc) as tc:
        with (
            tc.tile_pool(name="big", bufs=1) as big,
            tc.tile_pool(name="small", bufs=1) as small,
        ):
            # Cross-engine deps are tracked per-TILE at emission position, so
            # sum tiles have a single writer engine and each consumer is
            # emitted right after its producer.  The result tile res_t is
            # shared (cols 0:4 = R accums, 4:8 = negmeans); its cross-engine
            # writes land long after the reads of earlier columns, so the
            # tile-granular false deps it creates never stall anything.
            sm_t = small.tile([_P, nslot], f32)     # negw
            v_t = big.tile([_P, FREE], fv)
            a_t = big.tile([_P, LMAX], f16)     # ACT big-op out scratch
            d_t = big.tile([_P, LMAX], f16)     # DVE big-op out scratch
            z_t = big.tile([_P, LMAX], f16)     # zeros for the STT max
            asums = small.tile([_P, 2], f32)    # ACT: slots 0,3
            dsums = small.tile([_P, 2], f32)    # DVE: slots 1,2
            res_t = small.tile([_P, 2 * nslot], f32)

            # scalar queue: tiny metadata first — it staggers pair B's
            # descriptor drain behind pair A's, so pair A (whose consumers
            # are the critical left edge) gets the SDMA engines to itself;
            # pair B's consumers have >1us of slack.
            nc.scalar.dma_start(out=sm_t[:], in_=sm_d[:, :])
            nc.scalar.dma_start(out=v_t[:, OFF[2]:], in_=v_d[:, OFF[2]:])
            # sync queue: pair A (slots 0,1) as one DMA (single completion)
            nc.sync.dma_start(out=v_t[:, :OFF[2]], in_=v_d[:, :OFF[2]])
            negw = sm_t[:, 0:nslot]
            nc.scalar.activation(
                out=a_t[:, 0:1], in_=a_t[:, 0:1], func=Act.Relu,
                bias=0.0, scale=1.0,
            )
            nc.gpsimd.memset(z_t[:], 0.0)

            def sl(s):
                return v_t[:, OFF[s]:OFF[s + 1]]

            def emit_sum(s, eng, acc):
                if eng == "act":
                    nc.scalar.activation(
                        out=a_t[:, :Ls[s]], in_=sl(s), func=Act.Copy,
                        accum_out=acc)
                else:
                    nc.vector.tensor_scalar(
                        out=d_t[:, :Ls[s]], in0=sl(s), scalar1=1.0,
                        scalar2=None, op0=Alu.mult, op1=Alu.add,
                        accum_out=acc)

            def nm(s):
                return res_t[:, nslot + s:nslot + s + 1]

            def emit_nm(s, src):
                nc.vector.scalar_tensor_tensor(
                    out=nm(s), in0=src, scalar=1.0,
                    in1=negw[:, s:s + 1], op0=Alu.mult, op1=Alu.mult)

            def emit_dev(s, eng):
                acc = res_t[:, s:s + 1]
                if eng == "dve":
                    nc.vector.scalar_tensor_tensor(
                        out=d_t[:, :Ls[s]], in0=sl(s),
                        scalar=nm(s), in1=z_t[:, :Ls[s]],
                        op0=Alu.add, op1=Alu.max, accum_out=acc)
                else:
                    nc.scalar.activation(
                        out=a_t[:, :Ls[s]], in_=sl(s), func=Act.Relu,
                        bias=nm(s), scale=1.0, accum_out=acc)

            emit_sum(0, "act", asums[:, 0:1])
            emit_sum(1, "dve", dsums[:, 0:1])
            emit_nm(0, asums[:, 0:1])
            emit_nm(1, dsums[:, 0:1])
            emit_sum(3, "act", asums[:, 1:2])
            emit_dev(1, "act")
            emit_dev(0, "dve")
            emit_sum(2, "dve", dsums[:, 1:2])
            emit_nm(2, dsums[:, 1:2])
            emit_nm(3, asums[:, 1:2])
            emit_dev(2, "act")
            emit_dev(3, "dve")
            # ship [R | negmean] raw; the host applies 2*w*(R - npad*relu(nm))
            nc.sync.dma_start(out=out_d[:, :], in_=res_t[:])

    nc.finalize()
    return nc


_CACHE = {}


def _get_nc(key):
    if key not in _CACHE:
        _CACHE[key] = _build_nc(*key)
    return _CACHE[key]


def _pack(input, rows, cols, seg_ids, num_paths, vdt):
    """Host-side sharding: one image per core; each core's segments are
    sorted by length (ascending) and packed rank-ordered into a
    [128, sum(Ls)] grid, so each slot is padded only to its own max
    length and the first slots complete their DMA earliest.
    Returns -w to ship to the device and w/npad for the host combine,
    all derived from seg_ids alone."""
    from concourse import mybir

    B, C, H, W = input.shape
    ppi = num_paths // B  # paths (segments) per image
    npix = rows.shape[0]

    bnd = np.searchsorted(seg_ids, np.arange(num_paths + 1)).astype(np.int64)
    seg_lens = np.diff(bnd)
    nslot = int(np.ceil(ppi / _P))

    # per-core rank by length (shortest first, so slot 0 is smallest and
    # its DMA completes earliest)
    lens2 = seg_lens.reshape(B, ppi)
    order = np.argsort(lens2, axis=1, kind="stable")
    rank = np.empty_like(order)
    np.put_along_axis(rank, order, np.arange(ppi)[None, :].repeat(B, 0), 1)
    part = (rank % _P).ravel()
    slot = (rank // _P).ravel()

    # per-slot padded length, uniform across cores (same device program)
    pad = np.full((B, nslot * _P - ppi), 0, lens2.dtype)
    lens_sorted = np.take_along_axis(lens2, order, 1)
    lens_grid = np.concatenate([lens_sorted, pad], 1).reshape(B, nslot, _P)
    Ls = tuple(int(max(8, np.ceil(l / 8.0) * 8))
               for l in lens_grid.max(axis=(0, 2)))
    off = np.concatenate([[0], np.cumsum(Ls)]).astype(np.int64)
    FREE = int(off[-1])

    core = np.arange(num_paths) // ppi
    base = (core * _P + part) * FREE + off[slot]
    dest = np.repeat(base, seg_lens) + (
        np.arange(npix, dtype=np.int64) - np.repeat(bnd[:-1], seg_lens)
    )
    core_of = np.repeat(core, seg_lens)
    np_dt = mybir.dt.np({"f16": mybir.dt.float16,
                         "f8": mybir.dt.float8e4}[vdt])
    v_p = np.zeros(B * _P * FREE, np_dt)
    v_p[dest] = input[core_of, 0, rows, cols]

    counts = np.zeros((B, _P, nslot), np.float32)
    counts[core, part, slot] = seg_lens
    w = 1.0 / np.maximum(counts, 1.0)
    npad = np.float32(np.array(Ls))[None, None, :] - counts
    return (v_p.reshape(B, _P, FREE), np.ascontiguousarray(-w), w, npad,
            nslot, Ls)


def kernel(input, rows, cols, seg_ids, _trace=False, _num_paths=_NUM_PATHS,
           _vdt="f8", _sums_eng=None, _dev_eng=None, _split_dma=False):
    from concourse.bass_utils import run_bass_kernel_spmd

    input = np.ascontiguousarray(np.asarray(input, np.float32))
    rows = np.ascontiguousarray(np.asarray(rows, np.int32))
    cols = np.ascontiguousarray(np.asarray(cols, np.int32))
    seg_ids = np.ascontiguousarray(np.asarray(seg_ids, np.int32))
    B, C, H, W = input.shape

    v_p, sm, w, npad, nslot, Ls = _pack(
        input, rows, cols, seg_ids, _num_paths, _vdt)
    sums_eng = tuple(_sums_eng) if _sums_eng else ("act", "dve", "dve", "act")
    dev_eng = tuple(_dev_eng) if _dev_eng else ("dve", "act", "act", "dve")
    nc = _get_nc((nslot, Ls, _vdt, sums_eng, dev_eng, _split_dma))
    in_maps = [{"vP": v_p[i], "smP": sm[i]} for i in range(B)]
    res = run_bass_kernel_spmd(nc, in_maps, core_ids=list(range(B)), trace=_trace)
    total = 0.0
    for i, r in enumerate(res.results):
        o = np.asarray(r["out"], np.float32)
        R, negmean = o[:, :nslot], o[:, nslot:]
        contrib = 2.0 * w[i] * (R - npad[i] * np.maximum(negmean, 0.0))
        total += float(contrib.sum())
    out = np.float32(total / B)
    if _trace:
        return out, res
    return out


# revision 45
# speedup vs baseline: 1.0152x; 1.0152x over previous
"""CIGLoss (segment_reduce) Trainium2 kernel.

Strategy (data-parallel over batch, per the sharding hint):
  - Each of the 8 NeuronCores owns one image and that image's pixel list
    (segments are image-local: seg // 500 == image).
  - Host-side sharding packs each image's ~500 segments into a
    [128 partitions, NSLOT slots, L] padded grid (one whole segment per
    slot), values cast to fp16/fp8 (loss tolerance 2e-2 >> cast error).
    Pad entries are 0.  The value lookup input[b,0,row,col] happens
    during host packing (this toolchain's walrus mis-lowers per-element
    indirect DMA — verified by hardware probes in a previous session).
  - Per-segment counts are metadata (a function of seg_ids only); the
    host ships -w = -1/max(count,1) as a tiny f32 tensor and keeps
    w/npad for the final host-side combine.
  - On device, Sum_real |v-m| == 2*Sum_real relu(v-m) (real deviations
    sum to ~0), and pads (v=0) contribute relu(-m) each:
        sums_s  = accum_add(v_s)          ACT Copy-accum / DVE TS-accum
        negmean = -sums*w                 tiny DVE op
        R_s     = accum_add(relu(v_s + negmean))   ACT Relu-accum /
                                                   fused DVE STT-accum
    All accum ops run at 1x, so the 8 big passes are split evenly
    between the ACT and DVE engines (each slot's sum and dev on
    opposite engines), with single-writer-engine sum tiles and
    consumers emitted right after producers (cross-engine deps are
    tracked at tile granularity by emission position).  Both engines
    accumulate [R | negmean] into one shared result tile that is
    DMA'd out raw, so there is no device tail after the last dev op.
  - Host applies contrib = 2*w*(R - npad*relu(negmean)) per slot, sums
    the 8 cores' partials, and divides by B.
"""

import numpy as np

_NUM_PATHS = 4000
_P = 128  # SBUF partitions


def _build_nc(nslot: int, Ls: tuple, vdt: str, sums_eng: tuple, dev_eng: tuple,
              split_dma: bool):
    import concourse.bacc as bacc
    import concourse.tile as tile
    from concourse import mybir

    f32 = mybir.dt.float32
    fv = {"f16": mybir.dt.float16, "f8": mybir.dt.float8e4}[vdt]
    f16 = mybir.dt.float16
    Alu = mybir.AluOpType
    Act = mybir.ActivationFunctionType
    FREE = sum(Ls)
    OFF = [0]
    for l in Ls:
        OFF.append(OFF[-1] + l)
    LMAX = max(Ls)

    nc = bacc.Bacc("TRN2", debug=False)
    v_d = nc.dram_tensor("vP", [_P, FREE], fv, kind="ExternalInput")
    sm_d = nc.dram_tensor("smP", [_P, nslot], f32, kind="ExternalInput")
    out_d = nc.dram_tensor("out", [_P, 2 * nslot], f32, kind="ExternalOutput")

    assert nslot == 4 and sums_eng == ("act", "dve", "dve", "act") \
        and dev_eng == ("dve", "act", "act", "dve")
    with tile.TileContext(nc) as tc:
        with (
            tc.tile_pool(name="big", bufs=1) as big,
            tc.tile_pool(name="small", bufs=1) as small,
        ):
            # Cross-engine deps are tracked per-TILE at emission position, so
            # sum tiles have a single writer engine and each consumer is
            # emitted right after its producer.  The result tile res_t is
            # shared (cols 0:4 = R accums, 4:8 = negmeans); its cross-engine
            # writes land long after the reads of earlier columns, so the
            # tile-granular false deps it creates never stall anything.
            sm_t = small.tile([_P, nslot], f32)     # negw
            v_t = big.tile([_P, FREE], fv)
            a_t = big.tile([_P, LMAX], f16)     # ACT big-op out scratch
            d_t = big.tile([_P, LMAX], f16)     # DVE big-op out scratch
            z_t = big.tile([_P, LMAX], f16)     # zeros for the STT max
            res_t = small.tile([_P, 2 * nslot], f32)

            # scalar queue: tiny metadata first — it staggers pair B's
            # descriptor drain behind pair A's, so pair A (whose consumers
            # are the critical left edge) gets the SDMA engines to itself;
            # pair B's consumers have >1us of slack.
            nc.scalar.dma_start(out=sm_t[:], in_=sm_d[:, :])
            nc.scalar.dma_start(out=v_t[:, OFF[2]:], in_=v_d[:, OFF[2]:])
            # sync queue: pair A (slots 0,1) as one DMA (single completion)
            nc.sync.dma_start(out=v_t[:, :OFF[2]], in_=v_d[:, :OFF[2]])
            negw = sm_t[:, 0:nslot]
            nc.scalar.activation(
                out=a_t[:, 0:1], in_=a_t[:, 0:1], func=Act.Relu,
                bias=0.0, scale=1.0,
            )
            nc.gpsimd.memset(z_t[:], 0.0)

            def sl(s):
                return v_t[:, OFF[s]:OFF[s + 1]]

            def nm(s):
                return res_t[:, nslot + s:nslot + s + 1]

            def emit_sum(s, eng):
                # accumulate Sum(v * -w) = negmean directly (scale fused)
                if eng == "act":
                    nc.scalar.activation(
                        out=a_t[:, :Ls[s]], in_=sl(s), func=Act.Copy,
                        scale=negw[:, s:s + 1], accum_out=nm(s))
                else:
                    nc.vector.tensor_scalar(
                        out=d_t[:, :Ls[s]], in0=sl(s),
                        scalar1=negw[:, s:s + 1], scalar2=None,
                        op0=Alu.mult, op1=Alu.add, accum_out=nm(s))

            def emit_dev(s, eng):
                acc = res_t[:, s:s + 1]
                if eng == "dve":
                    nc.vector.scalar_tensor_tensor(
                        out=d_t[:, :Ls[s]], in0=sl(s),
                        scalar=nm(s), in1=z_t[:, :Ls[s]],
                        op0=Alu.add, op1=Alu.max, accum_out=acc)
                else:
                    nc.scalar.activation(
                        out=a_t[:, :Ls[s]], in_=sl(s), func=Act.Relu,
                        bias=nm(s), scale=1.0, accum_out=acc)

            emit_sum(0, "act")
            emit_sum(1, "dve")
            emit_sum(3, "act")
            emit_dev(0, "dve")
            emit_dev(1, "act")
            emit_sum(2, "dve")
            emit_dev(3, "dve")
            emit_dev(2, "act")
            # ship [R | negmean] raw; the host applies 2*w*(R - npad*relu(nm))
            nc.sync.dma_start(out=out_d[:, :], in_=res_t[:])

    nc.finalize()
    return nc


_CACHE = {}


def _get_nc(key):
    if key not in _CACHE:
        _CACHE[key] = _build_nc(*key)
    return _CACHE[key]


def _pack(input, rows, cols, seg_ids, num_paths, vdt):
    """Host-side sharding: one image per core; each core's segments are
    sorted by length (ascending) and packed rank-ordered into a
    [128, sum(Ls)] grid, so each slot is padded only to its own max
    length and the first slots complete their DMA earliest.
    Returns -w to ship to the device and w/npad for the host combine,
    all derived from seg_ids alone."""
    from concourse import mybir

    B, C, H, W = input.shape
    ppi = num_paths // B  # paths (segments) per image
    npix = rows.shape[0]

    bnd = np.searchsorted(seg_ids, np.arange(num_paths + 1)).astype(np.int64)
    seg_lens = np.diff(bnd)
    nslot = int(np.ceil(ppi / _P))

    # per-core rank by length (shortest first, so slot 0 is smallest and
    # its DMA completes earliest)
    lens2 = seg_lens.reshape(B, ppi)
    order = np.argsort(lens2, axis=1, kind="stable")
    rank = np.empty_like(order)
    np.put_along_axis(rank, order, np.arange(ppi)[None, :].repeat(B, 0), 1)
    part = (rank % _P).ravel()
    slot = (rank // _P).ravel()

    # per-slot padded length, uniform across cores (same device program)
    pad = np.full((B, nslot * _P - ppi), 0, lens2.dtype)
    lens_sorted = np.take_along_axis(lens2, order, 1)
    lens_grid = np.concatenate([lens_sorted, pad], 1).reshape(B, nslot, _P)
    Ls = tuple(int(max(8, np.ceil(l / 8.0) * 8))
               for l in lens_grid.max(axis=(0, 2)))
    off = np.concatenate([[0], np.cumsum(Ls)]).astype(np.int64)
    FREE = int(off[-1])

    core = np.arange(num_paths) // ppi
    base = (core * _P + part) * FREE + off[slot]
    dest = np.repeat(base, seg_lens) + (
        np.arange(npix, dtype=np.int64) - np.repeat(bnd[:-1], seg_lens)
    )
    core_of = np.repeat(core, seg_lens)
    np_dt = mybir.dt.np({"f16": mybir.dt.float16,
                         "f8": mybir.dt.float8e4}[vdt])
    v_p = np.zeros(B * _P * FREE, np_dt)
    v_p[dest] = input[core_of, 0, rows, cols]

    counts = np.zeros((B, _P, nslot), np.float32)
    counts[core, part, slot] = seg_lens
    w = 1.0 / np.maximum(counts, 1.0)
    npad = np.float32(np.array(Ls))[None, None, :] - counts
    return (v_p.reshape(B, _P, FREE), np.ascontiguousarray(-w), w, npad,
            nslot, Ls)


def kernel(input, rows, cols, seg_ids, _trace=False, _num_paths=_NUM_PATHS,
           _vdt="f8", _sums_eng=None, _dev_eng=None, _split_dma=False):
    from concourse.bass_utils import run_bass_kernel_spmd

    input = np.ascontiguousarray(np.asarray(input, np.float32))
    rows = np.ascontiguousarray(np.asarray(rows, np.int32))
    cols = np.ascontiguousarray(np.asarray(cols, np.int32))
    seg_ids = np.ascontiguousarray(np.asarray(seg_ids, np.int32))
    B, C, H, W = input.shape

    v_p, sm, w, npad, nslot, Ls = _pack(
        input, rows, cols, seg_ids, _num_paths, _vdt)
    sums_eng = tuple(_sums_eng) if _sums_eng else ("act", "dve", "dve", "act")
    dev_eng = tuple(_dev_eng) if _dev_eng else ("dve", "act", "act", "dve")
    nc = _get_nc((nslot, Ls, _vdt, sums_eng, dev_eng, _split_dma))
    in_maps = [{"vP": v_p[i], "smP": sm[i]} for i in range(B)]
    res = run_bass_kernel_spmd(nc, in_maps, core_ids=list(range(B)), trace=_trace)
    total = 0.0
    for i, r in enumerate(res.results):
        o = np.asarray(r["out"], np.float32)
        R, negmean = o[:, :nslot], o[:, nslot:]
        contrib = 2.0 * w[i] * (R - npad[i] * np.maximum(negmean, 0.0))
        total += float(contrib.sum())
    out = np.float32(total / B)
    if _trace:
        return out, res
    return out


# revision 46
# speedup vs baseline: 1.0235x; 1.0081x over previous
"""CIGLoss (segment_reduce) Trainium2 kernel.

Strategy (data-parallel over batch, per the sharding hint):
  - Each of the 8 NeuronCores owns one image and that image's pixel list
    (segments are image-local: seg // 500 == image).
  - Host-side sharding packs each image's ~500 segments into a
    [128 partitions, NSLOT slots, L] padded grid (one whole segment per
    slot), values cast to fp16/fp8 (loss tolerance 2e-2 >> cast error).
    Pad entries are 0.  The value lookup input[b,0,row,col] happens
    during host packing (this toolchain's walrus mis-lowers per-element
    indirect DMA — verified by hardware probes in a previous session).
  - Per-segment counts are metadata (a function of seg_ids only); the
    host ships -w = -1/max(count,1) as a tiny f32 tensor and keeps
    w/npad for the final host-side combine.
  - On device, Sum_real |v-m| == 2*Sum_real relu(v-m) (real deviations
    sum to ~0), and pads (v=0) contribute relu(-m) each:
        negmean = accum_add(v_s * -w)     scale fused into the sum pass
                                          (ACT Copy-accum w/ AP scale /
                                           DVE TS-accum w/ AP scalar)
        R_s     = accum_add(relu(v_s + negmean))   ACT Relu-accum /
                                                   fused DVE STT-accum
    All accum ops run at 1x, so the 8 big passes are split evenly
    between the ACT and DVE engines (each slot's sum and dev on
    opposite engines), with single-writer-engine sum tiles and
    consumers emitted right after producers (cross-engine deps are
    tracked at tile granularity by emission position).  Both engines
    accumulate [R | negmean] into one shared result tile that is
    DMA'd out raw, so there is no device tail after the last dev op.
  - Host applies contrib = 2*w*(R - npad*relu(negmean)) per slot, sums
    the 8 cores' partials, and divides by B.
"""

import numpy as np

_NUM_PATHS = 4000
_P = 128  # SBUF partitions


def _build_nc(nslot: int, Ls: tuple, vdt: str, sums_eng: tuple, dev_eng: tuple,
              split_dma: bool):
    import concourse.bacc as bacc
    import concourse.tile as tile
    from concourse import mybir

    f32 = mybir.dt.float32
    fv = {"f16": mybir.dt.float16, "f8": mybir.dt.float8e4}[vdt]
    f16 = mybir.dt.float16
    Alu = mybir.AluOpType
    Act = mybir.ActivationFunctionType
    FREE = sum(Ls)
    OFF = [0]
    for l in Ls:
        OFF.append(OFF[-1] + l)
    LMAX = max(Ls)

    nc = bacc.Bacc("TRN2", debug=False)
    v_d = nc.dram_tensor("vP", [_P, FREE], fv, kind="ExternalInput")
    sm_d = nc.dram_tensor("smP", [_P, nslot], f32, kind="ExternalInput")
    out_d = nc.dram_tensor("out", [_P, 2 * nslot], f32, kind="ExternalOutput")

    assert nslot == 4 and sums_eng == ("act", "dve", "dve", "act") \
        and dev_eng == ("dve", "act", "act", "dve")
    with tile.TileContext(nc) as tc:
        with (
            tc.tile_pool(name="big", bufs=1) as big,
            tc.tile_pool(name="small", bufs=1) as small,
        ):
            # Cross-engine deps are tracked per-TILE at emission position, so
            # sum tiles have a single writer engine and each consumer is
            # emitted right after its producer.  The result tile res_t is
            # shared (cols 0:4 = R accums, 4:8 = negmeans); its cross-engine
            # writes land long after the reads of earlier columns, so the
            # tile-granular false deps it creates never stall anything.
            sm_t = small.tile([_P, nslot], f32)     # negw
            v_t = big.tile([_P, FREE], fv)
            a_t = big.tile([_P, LMAX], f16)     # ACT big-op out scratch
            d_t = big.tile([_P, LMAX], f16)     # DVE big-op out scratch
            z_t = big.tile([_P, LMAX], f16)     # zeros for the STT max
            res_t = small.tile([_P, 2 * nslot], f32)

            # scalar queue: tiny metadata first — it staggers pair B's
            # descriptor drain behind pair A's, so pair A (whose consumers
            # are the critical left edge) gets the SDMA engines to itself;
            # pair B's consumers have >1us of slack.
            nc.scalar.dma_start(out=sm_t[:], in_=sm_d[:, :])
            nc.scalar.dma_start(out=v_t[:, OFF[2]:], in_=v_d[:, OFF[2]:])
            # sync queue: pair A (slots 0,1) as one DMA (single completion)
            nc.sync.dma_start(out=v_t[:, :OFF[2]], in_=v_d[:, :OFF[2]])
            negw = sm_t[:, 0:nslot]
            nc.scalar.activation(
                out=a_t[:, 0:1], in_=a_t[:, 0:1], func=Act.Relu,
                bias=0.0, scale=1.0,
            )
            nc.gpsimd.memset(z_t[:], 0.0)

            def sl(s):
                return v_t[:, OFF[s]:OFF[s + 1]]

            def nm(s):
                return res_t[:, nslot + s:nslot + s + 1]

            def emit_sum(s, eng):
                # accumulate Sum(v * -w) = negmean directly (scale fused)
                if eng == "act":
                    nc.scalar.activation(
                        out=a_t[:, :Ls[s]], in_=sl(s), func=Act.Copy,
                        scale=negw[:, s:s + 1], accum_out=nm(s))
                else:
                    nc.vector.tensor_scalar(
                        out=d_t[:, :Ls[s]], in0=sl(s),
                        scalar1=negw[:, s:s + 1], scalar2=None,
                        op0=Alu.mult, op1=Alu.add, accum_out=nm(s))

            def emit_dev(s, eng):
                acc = res_t[:, s:s + 1]
                if eng == "dve":
                    nc.vector.scalar_tensor_tensor(
                        out=d_t[:, :Ls[s]], in0=sl(s),
                        scalar=nm(s), in1=z_t[:, :Ls[s]],
                        op0=Alu.add, op1=Alu.max, accum_out=acc)
                else:
                    nc.scalar.activation(
                        out=a_t[:, :Ls[s]], in_=sl(s), func=Act.Relu,
                        bias=nm(s), scale=1.0, accum_out=acc)

            emit_sum(0, "act")
            emit_sum(1, "dve")
            emit_sum(3, "act")
            emit_dev(0, "dve")
            emit_dev(1, "act")
            emit_sum(2, "dve")
            emit_dev(3, "dve")
            emit_dev(2, "act")
            # ship [R | negmean] raw; the host applies 2*w*(R - npad*relu(nm))
            nc.sync.dma_start(out=out_d[:, :], in_=res_t[:])

    nc.finalize()
    return nc


_CACHE = {}


def _get_nc(key):
    if key not in _CACHE:
        _CACHE[key] = _build_nc(*key)
    return _CACHE[key]


def _pack(input, rows, cols, seg_ids, num_paths, vdt):
    """Host-side sharding: one image per core; each core's segments are
    sorted by length (ascending) and packed rank-ordered into a
    [128, sum(Ls)] grid, so each slot is padded only to its own max
    length and the first slots complete their DMA earliest.
    Returns -w to ship to the device and w/npad for the host combine,
    all derived from seg_ids alone."""
    from concourse import mybir

    B, C, H, W = input.shape
    ppi = num_paths // B  # paths (segments) per image
    npix = rows.shape[0]

    bnd = np.searchsorted(seg_ids, np.arange(num_paths + 1)).astype(np.int64)
    seg_lens = np.diff(bnd)
    nslot = int(np.ceil(ppi / _P))

    # per-core rank by length (shortest first, so slot 0 is smallest and
    # its DMA completes earliest)
    lens2 = seg_lens.reshape(B, ppi)
    order = np.argsort(lens2, axis=1, kind="stable")
    rank = np.empty_like(order)
    np.put_along_axis(rank, order, np.arange(ppi)[None, :].repeat(B, 0), 1)
    part = (rank % _P).ravel()
    slot = (rank // _P).ravel()

    # per-slot padded length, uniform across cores (same device program)
    pad = np.full((B, nslot * _P - ppi), 0, lens2.dtype)
    lens_sorted = np.take_along_axis(lens2, order, 1)
    lens_grid = np.concatenate([lens_sorted, pad], 1).reshape(B, nslot, _P)
    Ls = tuple(int(max(8, np.ceil(l / 8.0) * 8))
               for l in lens_grid.max(axis=(0, 2)))
    off = np.concatenate([[0], np.cumsum(Ls)]).astype(np.int64)
    FREE = int(off[-1])

    core = np.arange(num_paths) // ppi
    base = (core * _P + part) * FREE + off[slot]
    dest = np.repeat(base, seg_lens) + (
        np.arange(npix, dtype=np.int64) - np.repeat(bnd[:-1], seg_lens)
    )
    core_of = np.repeat(core, seg_lens)
    np_dt = mybir.dt.np({"f16": mybir.dt.float16,
                         "f8": mybir.dt.float8e4}[vdt])
    v_p = np.zeros(B * _P * FREE, np_dt)
    v_p[dest] = input[core_of, 0, rows, cols]

    counts = np.zeros((B, _P, nslot), np.float32)
    counts[core, part, slot] = seg_lens
    w = 1.0 / np.maximum(counts, 1.0)
    npad = np.float32(np.array(Ls))[None, None, :] - counts
    return (v_p.reshape(B, _P, FREE), np.ascontiguousarray(-w), w, npad,
            nslot, Ls)


def kernel(input, rows, cols, seg_ids, _trace=False, _num_paths=_NUM_PATHS,
           _vdt="f8", _sums_eng=None, _dev_eng=None, _split_dma=False):
    from concourse.bass_utils import run_bass_kernel_spmd

    input = np.ascontiguousarray(np.asarray(input, np.float32))
    rows = np.ascontiguousarray(np.asarray(rows, np.int32))
    cols = np.ascontiguousarray(np.asarray(cols, np.int32))
    seg_ids = np.ascontiguousarray(np.asarray(seg_ids, np.int32))
    B, C, H, W = input.shape

    v_p, sm, w, npad, nslot, Ls = _pack(
        input, rows, cols, seg_ids, _num_paths, _vdt)
    sums_eng = tuple(_sums_eng) if _sums_eng else ("act", "dve", "dve", "act")
    dev_eng = tuple(_dev_eng) if _dev_eng else ("dve", "act", "act", "dve")
    nc = _get_nc((nslot, Ls, _vdt, sums_eng, dev_eng, _split_dma))
    in_maps = [{"vP": v_p[i], "smP": sm[i]} for i in range(B)]
    res = run_bass_kernel_spmd(nc, in_maps, core_ids=list(range(B)), trace=_trace)
    total = 0.0
    for i, r in enumerate(res.results):
        o = np.asarray(r["out"], np.float32)
        R, negmean = o[:, :nslot], o[:, nslot:]
        contrib = 2.0 * w[i] * (R - npad[i] * np.maximum(negmean, 0.0))
        total += float(contrib.sum())
    out = np.float32(total / B)
    if _trace:
        return out, res
    return out
